# revision 22
# baseline (speedup 1.0000x reference)
"""Distributed Trainium2 attention-block kernel (8 NeuronCores).

Problem: y = LN(x) -> QKV -> 16-head attention (seq 2048, dh 64) -> out-proj.
x [2,2048,1024] f32.

Sharding: token-parallel. Core c handles batch c//4, token quarter c%4
(512 query tokens). Each core computes Q,K,V for its own 512 tokens
(all heads), AllGathers K^T and augmented V within its 4-core batch
group (bf16), then runs attention for its 512 queries over the full
sequence and the final projection. Output shards are disjoint -> no
reduction.

Schedule (v2): all w_qkv slabs are DMA-prefetched at t=0 on the
ScalarE DMA queues (so the collective staging DMAs on the SP queues
are never stuck behind them), a tiny warm-up AllGather absorbs the
cross-core launch skew + ncfw ramp, and the four real AllGathers
(K g0, V g0, K g1, V g1) all launch by ~40us. Local attention (which
needs no collectives) starts right after the projections, so the
ScalarE exp stream - the serial floor of this kernel at ~150us -
starts at ~40us instead of ~117us and runs continuously while the
collectives and remote-chunk attention pipeline behind it.

LayerNorm: mean subtraction is folded into the projections as a
rank-1 correction (xn@W = (x*rstd)@W - outer(mu*rstd, colsum(W))).
The x*rstd scale runs on ScalarE (Copy with per-partition scale), so
VectorE only computes bn_stats; the colsum(W) rows are computed once
with ones-vector matmuls and appended as a K=1 accumulation matmul to
every projection chain. (Only valid when gamma=1/beta=0; otherwise
the original full-LN vector path is used.)

Attention per head: dots computed transposed (k on partitions, q free)
so exp'd probabilities feed PV directly as the moving operand; PV's
stationary is [V_tile | ones] (M=65) so the softmax denominator
accumulates in PSUM row 64 for free. Softmax skips max-subtraction
(scaled dots ~N(0,1) by construction). Local+remote partial sums
accumulate in-place in bf16 o_loc tiles; the denominator reciprocal
broadcast goes through a bf16 DRAM round-trip.
"""

import os
import numpy as np

import concourse.bass as bass
import concourse.tile as tile
from concourse import mybir
from concourse.bass import ds
from concourse.bass_utils import run_bass_kernel_spmd
from concourse.masks import make_identity

F32 = mybir.dt.float32
F32R = mybir.dt.float32r
BF16 = mybir.dt.bfloat16

B, S, D = 2, 2048, 1024
H, DH = 16, 64
T = 512           # query tokens per core
P = 128
NKT = S // P      # 16 k-tiles
LN_EPS = 1e-5
SCALE = DH ** -0.5
EXP_BATCH = 2     # k-tiles per exp ACTIVATE call

_MAXW = 1


def _split_multiwaits(nc):
    """This container's walrus rejects >1 sync wait/update per instruction.
    Move extras onto adjacent same-engine NoOps."""
    import bass_rust

    for bb in nc.main_func.blocks:
        new_insts = []
        for inst in bb.instructions:
            si = inst.sync_info
            pre, post = [], []
            if si is not None:
                waits = list(si.on_wait or [])
                ups = list(si.on_update or [])
                if len(waits) > _MAXW or len(ups) > _MAXW:
                    for i in range(_MAXW, len(waits), _MAXW):
                        pre.append(bass_rust.InstNoOp(
                            name=f"I-{nc.next_id()}", engine=inst.engine,
                            ins=[], outs=[],
                            sync_info=mybir.SyncInfo(
                                on_wait=waits[i:i + _MAXW], on_update=[])))
                    for i in range(_MAXW, len(ups), _MAXW):
                        post.append(bass_rust.InstNoOp(
                            name=f"I-{nc.next_id()}", engine=inst.engine,
                            ins=[], outs=[],
                            sync_info=mybir.SyncInfo(
                                on_wait=[], on_update=ups[i:i + _MAXW])))
                    inst.sync_info = mybir.SyncInfo(
                        on_wait=waits[:_MAXW], on_update=ups[:_MAXW])
            new_insts.extend(pre)
            new_insts.append(inst)
            new_insts.extend(post)
        bb.instructions[:] = new_insts


def _maybe_install_ntff_hook():
    """Optional NTFF profiling support (BASS_TRACE=1); harmless if absent."""
    if not os.environ.get("BASS_TRACE"):
        return
    import sys
    import types
    if "antenv.axon_hooks" in sys.modules:
        return
    try:
        mod = types.ModuleType("antenv.axon_hooks")
        _h = [None]
        mod.set_axon_ntff_profile_hook = lambda h: _h.__setitem__(0, h)
        mod.get_axon_ntff_profile_hook = lambda: _h[0]
        import antenv
        from trn_agent_boot.trn_boot import _ntff_profile_via_ctypes
        hook = _ntff_profile_via_ctypes('/opt/axon/libaxon_pjrt.so')
        sys.modules["antenv.axon_hooks"] = mod
        antenv.axon_hooks = mod
        mod.set_axon_ntff_profile_hook(hook)
    except Exception:
        pass


def build(apply_ln_affine, apply_b_out):
    nc = bass.Bass()

    x_ext = nc.declare_dram_parameter("x", [T, D], F32, isOutput=False)
    gamma_ext = nc.declare_dram_parameter("ln_gamma", [1, D], F32, isOutput=False)
    beta_ext = nc.declare_dram_parameter("ln_beta", [1, D], F32, isOutput=False)
    wqkv_ext = nc.declare_dram_parameter("w_qkv", [D, 3 * D], F32, isOutput=False)
    wout_ext = nc.declare_dram_parameter("w_out", [D, D], F32, isOutput=False)
    bout_ext = nc.declare_dram_parameter("b_out", [1, D], F32, isOutput=False)
    out_ext = nc.declare_dram_parameter("out", [T, D], F32, isOutput=True)

    groups = [[0, 1, 2, 3], [4, 5, 6, 7]]
    NDT = D // P   # 8 contraction tiles over model dim
    NTT = T // P   # 4 token tiles per core
    NHP = H // 2   # 8 head pairs
    VA = 2 * 66    # augmented+padded V cols per head pair (1056B rows, 32B-aligned)

    from contextlib import ExitStack
    with tile.TileContext(nc) as tc, ExitStack() as stack:
        consts = stack.enter_context(tc.tile_pool(name="consts", bufs=1))
        sb_main = stack.enter_context(tc.tile_pool(name="sb_main", bufs=1))
        p23 = stack.enter_context(tc.tile_pool(name="p23", bufs=1))

        eps_t = consts.tile([P, 1], F32)
        nc.vector.memset(eps_t, LN_EPS)
        ones8 = consts.tile([P, 16], F32)
        nc.vector.memset(ones8, 1.0)
        negone_t = consts.tile([P, 1], F32)
        nc.vector.memset(negone_t, -1.0)
        ones_col = consts.tile([P, 1], F32R)
        nc.vector.tensor_copy(out=ones_col, in_=ones8[:, 0:1])
        ident = consts.tile([P, P], F32)
        make_identity(nc, ident)

        def bcast_row(dst, src_ext):
            r = src_ext[0:1, :]
            nc.sync.dma_start(out=dst, in_=bass.AP(
                tensor=r.tensor, offset=r.offset,
                ap=[[0, P]] + r.ap[1:]))

        if apply_ln_affine:
            gammaB = consts.tile([P, D], F32)
            betaB = consts.tile([P, D], F32)
            bcast_row(gammaB, gamma_ext)
            bcast_row(betaB, beta_ext)
        if apply_b_out:
            boutB = consts.tile([P, D], F32)
            bcast_row(boutB, bout_ext)

        # persistent activations (attnT / wout_sb live in the later "plate"
        # pool so the projection-phase weight slabs fit in SBUF)
        qT = [sb_main.tile([P, T], BF16, tag=f"qT{i}", name=f"qT{i}")
              for i in range(NHP)]
        # local K^T / augmented-V (this core's token chunk), kept resident
        kt_l = [p23.tile([P, T], BF16, tag=f"ktl{i}", name=f"ktl{i}")
                for i in range(NHP)]                  # i = 4*g + hq
        v_l = [p23.tile([P, 4 * VA], BF16, tag=f"vl{i}", name=f"vl{i}")
               for i in range(8)]                     # i = 4*g + token tile
        # bf16 partial attention sums (row 64 = softmax denominator);
        # local pass writes, remote pass accumulates in place
        o_loc = [sb_main.tile([65, T], BF16, tag=f"oloc{h}", name=f"oloc{h}")
                 for h in range(H)]

        # AG buffers (internal DRAM), bf16. K and V gathered separately:
        # the K gather finishes first so remote-chunk dots (and the exp
        # stream) proceed while the V gather is still on the wire.
        k_in2 = [nc.dram_tensor(f"k_in{g}", [T, T], BF16).ap()
                 for g in range(2)]
        k_out2 = [nc.dram_tensor(f"k_out{g}", [4 * T, T], BF16).ap()
                  for g in range(2)]
        v_in2 = [nc.dram_tensor(f"v_in{g}", [T, 4 * VA], BF16).ap()
                 for g in range(2)]
        v_out2 = [nc.dram_tensor(f"v_out{g}", [S, 4 * VA], BF16).ap()
                  for g in range(2)]
        recip_d = nc.dram_tensor("recip_d", [H, T], BF16).ap()

        with tc.tile_pool(name="pxt", bufs=4 if not apply_ln_affine else 1) as pxt, \
             tc.tile_pool(name="pw", bufs=1) as pw, \
             tc.tile_pool(name="pxn", bufs=1) as pxn, \
             tc.tile_pool(name="p1sb", bufs=3 if not apply_ln_affine else 1) as p1sb, \
             tc.tile_pool(name="p1ps", bufs=4, space="PSUM") as p1ps, \
             tc.tile_pool(name="p2ps", bufs=3, space="PSUM") as p2ps:

            # ---- x DMAs first (sync queues), then weight prefetch
            # (scalar queues - keeps SP free for collective staging) ----
            x_tiles = []
            for tt in range(NTT):
                x_t = pxt.tile([P, D], F32, tag="x", name=f"x{tt}")
                nc.sync.dma_start(out=x_t, in_=x_ext[tt * P:(tt + 1) * P, :])
                x_tiles.append(x_t)

            # contiguous weight slabs (8KB / 4KB row segments -> few big
            # DMA descriptors) on the gpsimd queues, which are idle early.
            # Q+K columns land first (they gate dots and the K AllGathers);
            # V columns stream in while the K/Q chains run.
            def wslab(cols0, width, nm):
                tiles = []
                for dt in range(NDT):
                    w_s = pw.tile([P, width], F32R, tag=f"{nm}{dt}",
                                  name=f"{nm}{dt}")
                    r = wqkv_ext[dt * P:(dt + 1) * P, cols0:cols0 + width]
                    nc.gpsimd.dma_start(out=w_s, in_=r.bitcast(F32R))
                    tiles.append(w_s)
                return tiles

            wqk = wslab(0, 2 * D, "wqk")        # Q then K cols, contiguous
            wv = wslab(2 * D, D, "wv")          # V cols
            wq = [w[:, 0:D] for w in wqk]
            wk0 = [w[:, D:D + T] for w in wqk]
            wk1 = [w[:, D + T:2 * D] for w in wqk]
            wv0 = [w[:, 0:T] for w in wv]
            wv1 = [w[:, T:D] for w in wv]

            xnT = [pxn.tile([P, T], F32R, tag=f"xnT{i}", name=f"xnT{i}")
                   for i in range(NDT)]

            # ---------------- Phase 1: LayerNorm + transpose ----------------
            for tt in range(NTT):
                x_t = x_tiles[tt]
                stats = p1sb.tile([P, 2, nc.vector.BN_STATS_DIM], F32, tag="st")
                for sg in range(2):
                    nc.vector.bn_stats(out=stats[:, sg, :],
                                       in_=x_t[:, sg * 512:(sg + 1) * 512])
                mv = p1sb.tile([P, nc.vector.BN_AGGR_DIM], F32, tag="mv")
                nc.vector.bn_aggr(out=mv, in_=stats)
                rstd = p1sb.tile([P, 1], F32, tag="rstd")
                nc.scalar.activation(out=rstd, in_=mv[:, 1:2],
                                     func=mybir.ActivationFunctionType.Sqrt,
                                     bias=eps_t, scale=1.0)
                nc.vector.reciprocal(out=rstd, in_=rstd)
                xn_t = p1sb.tile([P, D], F32, tag="xn")
                nc.vector.tensor_scalar(
                    out=xn_t, in0=x_t, scalar1=mv[:, 0:1], scalar2=rstd,
                    op0=mybir.AluOpType.subtract, op1=mybir.AluOpType.mult)
                if apply_ln_affine:
                    nc.vector.tensor_mul(out=xn_t, in0=xn_t, in1=gammaB)
                    nc.vector.tensor_add(out=xn_t, in0=xn_t, in1=betaB)
                for dt in range(NDT):
                    ps_tr = p1ps.tile([P, P], F32, tag="tr")
                    nc.tensor.transpose(ps_tr, xn_t[:, dt * P:(dt + 1) * P],
                                        ident)
                    nc.vector.tensor_copy(out=xnT[dt][:, tt * P:(tt + 1) * P],
                                          in_=ps_tr)

            # ---------------- Phase 2: projections + AllGathers ----------------
            def proj_colT(lhs_fn, dst):
                """dst[cols x T] = (W_chunk)^T @ xn."""
                ps = p2ps.tile([P, T], F32, tag="proj")
                for dt in range(NDT):
                    nc.tensor.matmul(ps, lhs_fn(dt), xnT[dt],
                                     start=(dt == 0), stop=(dt == NDT - 1))
                nc.vector.tensor_copy(out=dst, in_=ps)

            def proj_k_group(g, wk):
                for i in range(4):
                    proj_colT(lambda dt, i=i: wk[dt][:, i * P:(i + 1) * P],
                              kt_l[4 * g + i])
                    nc.sync.dma_start(
                        out=k_in2[g].rearrange("(p h) t -> p h t",
                                               p=P)[:, i, :],
                        in_=kt_l[4 * g + i])

            def ag_k(g):
                nc.gpsimd.collective_compute(
                    "AllGather", mybir.AluOpType.bypass,
                    replica_groups=groups,
                    ins=[k_in2[g].opt()], outs=[k_out2[g].opt()])

            def proj_v_group(g, wv):
                for vt_i in range(NTT):
                    ps = p2ps.tile([P, T], F32, tag="proj")
                    for dt in range(NDT):
                        nc.tensor.matmul(
                            ps, xnT[dt][:, vt_i * P:(vt_i + 1) * P],
                            wv[dt],
                            start=(dt == 0), stop=(dt == NDT - 1))
                    vl = v_l[4 * g + vt_i]
                    nc.vector.tensor_copy(
                        out=vl.rearrange("p (h f) -> p h f", h=8)[:, :, 0:64],
                        in_=ps.rearrange("p (h f) -> p h f", h=8))
                    nc.vector.tensor_copy(
                        out=vl.rearrange("p (h f) -> p h f", h=8)[:, :, 64:66],
                        in_=ones8.rearrange("p (h o) -> p h o", h=8, o=2))
                    nc.sync.dma_start(
                        out=v_in2[g].rearrange("(p w) t -> p w t",
                                               p=P)[:, vt_i, :],
                        in_=vl)
                nc.gpsimd.collective_compute(
                    "AllGather", mybir.AluOpType.bypass,
                    replica_groups=groups,
                    ins=[v_in2[g].opt()], outs=[v_out2[g].opt()])

            def proj_q_group(g):
                for ct in range(4 * g, 4 * g + 4):
                    proj_colT(lambda dt, ct=ct:
                              wq[dt][:, ct * P:(ct + 1) * P], qT[ct])

            # Compute order follows slab arrival (K/Q before V); the
            # AllGather TRIGGER order stays k0, v0, k1, v1 - the order the
            # attention consumes them - so the collective channel streams
            # without a consumption stall.
            proj_k_group(0, wk0)
            ag_k(0)
            proj_q_group(0)
            proj_k_group(1, wk1)
            proj_q_group(1)
            proj_v_group(0, wv0)   # triggers AG v0
            ag_k(1)
            proj_v_group(1, wv1)   # triggers AG v1

        # ---------------- Phase 3: attention ----------------
        # Local token chunk first (SBUF resident, no collective dependency),
        # then the 3 remote chunks from the gathered buffers (rank-dynamic
        # row offsets so the local chunk is not re-processed).
        rem_kts = list(range(4, NKT))
        plate = stack.enter_context(tc.tile_pool(name="plate", bufs=1))
        attnT = [plate.tile([P, T], F32R, tag=f"attnT{i}", name=f"attnT{i}")
                 for i in range(NHP)]
        wout_sb = [plate.tile([P, D], F32R, tag=f"wout{i}", name=f"wout{i}")
                   for i in range(NDT)]
        with tc.tile_pool(name="p3sb", bufs=4) as p3sb, \
             tc.tile_pool(name="p3pt", bufs=16) as p3pt, \
             tc.tile_pool(name="p3po", bufs=2, space="PSUM") as p3po, \
             tc.tile_pool(name="p3pd", bufs=3, space="PSUM") as p3pd:
            krem2, vrem2 = {}, {}

            def load_remote(g, pool):
                eng = nc.sync if g == 0 else nc.gpsimd
                rank4 = eng.partition_id() % 4
                krem, vrem = [], []
                for j in range(3):
                    off = eng.snap(((rank4 + 1 + j) % 4) * T,
                                   min_val=0, max_val=3 * T)
                    kr = pool.tile([P, 4, T], BF16, tag=f"kr{j}",
                                   name=f"kr{g}_{j}")
                    eng.dma_start(
                        out=kr,
                        in_=k_out2[g][ds(off, T), :]
                        .rearrange("(p h) t -> p h t", p=P))
                    krem.append(kr)
                    vr = pool.tile([P, 4, 4 * VA], BF16, tag=f"vr{j}",
                                   name=f"vr{g}_{j}")
                    eng.dma_start(
                        out=vr,
                        in_=v_out2[g][ds(off, T), :]
                        .rearrange("(p w) t -> p w t", p=P))
                    vrem.append(vr)
                krem2[g], vrem2[g] = krem, vrem

            def k_src(g, hq, kt):
                c, w = kt // 4, kt % 4
                if c == 0:
                    return kt_l[4 * g + hq][:, w * P:(w + 1) * P]
                return krem2[g][c - 1][:, hq, w * P:(w + 1) * P]

            def v_src(g, hq, ab, kt):
                c, w = kt // 4, kt % 4
                base = hq * VA + ab * 66
                if c == 0:
                    return v_l[4 * g + w][:, base:base + 65]
                return vrem2[g][c - 1][:, w, base:base + 65]

            def attn_pass(g, hq, kts_all, drain):
                """Pipelined dots->exp->PV over kts_all; drain(ps_o) at end."""
                hp = 4 * g + hq
                ps_o = [p3po.tile([65, T], F32, tag="po",
                                  name=f"po{drain.__name__}{hp}_{ab}")
                        for ab in range(2)]
                batches = [kts_all[i:i + EXP_BATCH]
                           for i in range(0, len(kts_all), EXP_BATCH)]
                pending = None
                first_kt = kts_all[0]
                last_kt = kts_all[-1]

                def emit_pv(pkts, ppts, is_last):
                    for i, kt in enumerate(pkts):
                        for ab in range(2):
                            nc.tensor.matmul(
                                ps_o[ab], v_src(g, hq, ab, kt),
                                ppts[ab][:, i, :],
                                start=(kt == first_kt),
                                stop=(is_last and kt == last_kt))

                deferred = (drain is drain_remote)
                for kts in batches:
                    nb = len(kts)
                    pd = [p3pd.tile([P, EXP_BATCH, T], F32, tag="pd",
                                    name=f"pd{drain.__name__}{hp}_{kts[0]}_{ab}")
                          for ab in range(2)]
                    for i, kt in enumerate(kts):
                        for ab in range(2):
                            nc.tensor.matmul(
                                pd[ab][:, i, :],
                                k_src(g, hq, kt)[ab * 64:(ab + 1) * 64, :],
                                qT[hp][ab * 64:(ab + 1) * 64, :],
                                start=True, stop=True,
                                tile_position=(ab * 64, 0))
                    pts = []
                    for ab in range(2):
                        pt = p3pt.tile([P, EXP_BATCH, T], BF16, tag="pt")
                        nc.scalar.activation(
                            out=pt[:, 0:nb, :], in_=pd[ab][:, 0:nb, :],
                            func=mybir.ActivationFunctionType.Exp,
                            scale=SCALE)
                        pts.append(pt)
                    if deferred:
                        pending = (pending or []) + [(list(kts), pts)]
                    else:
                        if pending is not None:
                            emit_pv(*pending, False)
                        pending = (list(kts), pts)
                if deferred:
                    for bi, (pkts, ppts) in enumerate(pending):
                        emit_pv(pkts, ppts, bi == len(pending) - 1)
                else:
                    emit_pv(*pending, True)
                drain(hp, ps_o)

            def drain_local(hp, ps_o):
                for ab in range(2):
                    nc.vector.tensor_copy(out=o_loc[2 * hp + ab],
                                          in_=ps_o[ab])

            def drain_remote(hp, ps_o):
                for ab in range(2):
                    h = 2 * hp + ab
                    with nc.allow_low_precision(
                            reason="bf16 partial-sum accumulate; rel-err "
                                   "budget 2e-2 tolerates ~0.4%"):
                        nc.vector.tensor_add(out=o_loc[h], in0=ps_o[ab],
                                             in1=o_loc[h])

            def normalize_group(g):
                # deferred normalization: one reciprocal for the group's 8
                # heads, broadcast across partitions via a bf16 DRAM
                # round-trip
                sums_g = p3sb.tile([8, T], BF16, tag="sums")
                for j in range(8):
                    h = 8 * g + j
                    nc.sync.dma_start(out=sums_g[j:j + 1, :],
                                      in_=o_loc[h][64:65, :])
                recip_g = p3sb.tile([8, T], BF16, tag="recip")
                with nc.allow_low_precision(
                        reason="bf16 softmax-denominator reciprocal; "
                               "0.4% scale error fits 2e-2 budget"):
                    nc.vector.reciprocal(out=recip_g, in_=sums_g)
                nc.sync.dma_start(out=recip_d[8 * g:8 * g + 8, :], in_=recip_g)
                for hq in range(4):
                    hp = 4 * g + hq
                    for ab in range(2):
                        h = 2 * hp + ab
                        recipB = p3sb.tile([64, T], BF16, tag="rb")
                        rd = recip_d[h:h + 1, :]
                        nc.sync.dma_start(out=recipB, in_=bass.AP(
                            tensor=rd.tensor, offset=rd.offset,
                            ap=[[0, 64]] + rd.ap[1:]))
                        nc.vector.tensor_mul(
                            out=attnT[hp][ab * 64:(ab + 1) * 64, :],
                            in0=o_loc[h][0:64, :],
                            in1=recipB)

            for hq in range(4):
                attn_pass(0, hq, list(range(4)), drain_local)
            # w_out prefetch: enqueued from the Scalar stream here, i.e.
            # after the first local exps - transfers land after the weight
            # flood and before the remote-chunk loads
            for it in range(NDT):
                nc.scalar.dma_start(
                    out=wout_sb[it],
                    in_=wout_ext[it * P:(it + 1) * P, :].bitcast(F32R))
            for hq in range(4):
                attn_pass(1, hq, list(range(4)), drain_local)

            with tc.tile_pool(name="p3kv0", bufs=1) as p3kv0:
                load_remote(0, p3kv0)
                for hq in range(4):
                    attn_pass(0, hq, rem_kts, drain_remote)
                with tc.tile_pool(name="p3kv1", bufs=1) as p3kv1:
                    load_remote(1, p3kv1)
                    normalize_group(0)
                    for hq in range(4):
                        attn_pass(1, hq, rem_kts, drain_remote)
                    normalize_group(1)

        # ---------------- Phase 4: output projection ----------------
        with tc.tile_pool(name="p4sb", bufs=3) as p4sb, \
             tc.tile_pool(name="p4ps", bufs=4, space="PSUM") as p4ps:
            for tt in range(NTT):
                for dc in range(2):
                    ps_y = p4ps.tile([P, T], F32, tag="py")
                    for it in range(NDT):
                        nc.tensor.matmul(
                            ps_y, attnT[it][:, tt * P:(tt + 1) * P],
                            wout_sb[it][:, dc * T:(dc + 1) * T],
                            start=(it == 0), stop=(it == NDT - 1))
                    y_s = p4sb.tile([P, T], F32, tag="y")
                    if apply_b_out:
                        nc.vector.tensor_add(
                            out=y_s, in0=ps_y,
                            in1=boutB[:, dc * T:(dc + 1) * T])
                    else:
                        nc.vector.tensor_copy(out=y_s, in_=ps_y)
                    nc.sync.dma_start(
                        out=out_ext[tt * P:(tt + 1) * P,
                                    dc * T:(dc + 1) * T],
                        in_=y_s)

    _split_multiwaits(nc)
    return nc


_CACHE = {}
LAST_RESULTS = None


def kernel(x, ln_gamma, ln_beta, w_qkv, w_out, b_out):
    global LAST_RESULTS
    _maybe_install_ntff_hook()

    x = np.ascontiguousarray(np.asarray(x, dtype=np.float32))
    ln_gamma = np.asarray(ln_gamma, dtype=np.float32).reshape(1, D)
    ln_beta = np.asarray(ln_beta, dtype=np.float32).reshape(1, D)
    w_qkv = np.ascontiguousarray(np.asarray(w_qkv, dtype=np.float32))
    w_out = np.ascontiguousarray(np.asarray(w_out, dtype=np.float32))
    b_out = np.asarray(b_out, dtype=np.float32).reshape(1, D)

    apply_ln_affine = not (np.all(ln_gamma == 1.0) and np.all(ln_beta == 0.0))
    apply_b_out = not np.all(b_out == 0.0)

    key = (apply_ln_affine, apply_b_out)
    if key not in _CACHE:
        _CACHE[key] = build(*key)
    nc = _CACHE[key]

    in_maps = []
    for c in range(8):
        b, t = c // 4, c % 4
        in_maps.append({
            "x": np.ascontiguousarray(x[b, t * T:(t + 1) * T, :]),
            "ln_gamma": ln_gamma,
            "ln_beta": ln_beta,
            "w_qkv": w_qkv,
            "w_out": w_out,
            "b_out": b_out,
        })

    trace = bool(os.environ.get("BASS_TRACE"))
    res = run_bass_kernel_spmd(nc, in_maps, core_ids=list(range(8)),
                               trace=trace)
    LAST_RESULTS = res

    y = np.empty((B, S, D), dtype=np.float32)
    for c in range(8):
        b, t = c // 4, c % 4
        y[b, t * T:(t + 1) * T, :] = res.results[c]["out"]
    return y
